# revision 7
# baseline (speedup 1.0000x reference)
"""Trainium2 Bass kernel for nn_EdgeConvolution (gnn_message_passing).

Math
----
Reference (B=2, N=512, C=128, U=128), adj binary {0,1}:
  masked[b,i,j,:]  = adj[b,i,j] * x[b,i,:]
  a_sel[b,i]       = adj[b,i, xidx[b,i]]
  edging[b,i,j,:]  = [ adj*x_i | adj*(a_sel - adj)*x_i ]
                   = adj[b,i,j] * [ x_i | (a_sel_i - 1)*x_i ]        (adj^2 = adj)
  out[b,i,j,:]     = relu(adj*(u_i + (a_sel_i-1)*v_i) + b),  u = x@W1, v = x@W2
So over j there are only two values per (b,i):
  z1_i = relu(u_i + (a_sel_i-1)*v_i + b)   (edges with adj=1, count k_i)
  z0   = relu(b)                            (edges with adj=0, count N-k_i)
  maxp_i   = max(1[k_i>0]*z1_i, 1[k_i<N]*z0)
  n_i      = k_i*1[any z1_i>0] + (N-k_i)*1[any z0>0]
  avgpool_i = [ k_i*x_i | k_i*(a_sel_i-1)*x_i ] / n_i
Per-core slab: 128 of the 1024 (b,i) rows; w/b replicated.
`n` is computed via a select between the two possible reciprocals
(1/(k+t2) vs 1/t2, t2=(N-k)*s0) so only one op depends on s1.
"""

import numpy as np

B, N, C, U = 2, 512, 128, 128
P = 128          # rows (b,i) per core == SBUF partitions
NCORES = 8
OUTF = U + 2 * C  # 384

_CACHE: dict = {}


def _build_nc():
    import concourse.bacc as bacc
    import concourse.bass as bass
    import concourse.mybir as mybir
    import concourse.tile as tile

    f32 = mybir.dt.float32
    i32 = mybir.dt.int32
    Alu = mybir.AluOpType
    AX = mybir.AxisListType.X

    nc = bacc.Bacc("TRN2", target_bir_lowering=False, debug=False,
                   num_devices=NCORES)

    adj_d = nc.dram_tensor("adj", [P, N], f32, kind="ExternalInput")
    x_d = nc.dram_tensor("x", [P, C], f32, kind="ExternalInput")
    xT_d = nc.dram_tensor("xT", [C, P], f32, kind="ExternalInput")
    xidx_d = nc.dram_tensor("xidx", [P, 1], i32, kind="ExternalInput")
    w_d = nc.dram_tensor("w", [2 * C, U], f32, kind="ExternalInput")
    b_d = nc.dram_tensor("b", [1, U], f32, kind="ExternalInput")
    out_d = nc.dram_tensor("out", [P, OUTF], f32, kind="ExternalOutput")

    with tile.TileContext(nc) as tc:
        with tc.tile_pool(name="sb", bufs=1) as pool, \
             tc.tile_pool(name="ps", bufs=1, space="PSUM") as psum:
            # ---- input DMAs: adj first (it gates the row stats), split
            # across the two HWDGE rings (sync=SP, scalar=ACT) ----
            adj_t = pool.tile([P, N], f32)
            nc.sync.dma_start(out=adj_t[:], in_=adj_d.ap())
            xidx_i = pool.tile([P, 1], i32)
            nc.sync.dma_start(out=xidx_i[:], in_=xidx_d.ap())
            bfull = pool.tile([P, U], f32)
            b_ap = b_d.ap()
            nc.sync.dma_start(
                out=bfull[:],
                in_=bass.AP(b_ap.tensor, b_ap.offset, [[0, P], [1, U]]))

            xT = pool.tile([C, P], f32)
            nc.scalar.dma_start(out=xT[:], in_=xT_d.ap())
            wcat = pool.tile([P, 2 * U], f32)  # [c, U|U] = [W1 | W2]
            nc.scalar.dma_start(out=wcat[:, 0:U], in_=w_d.ap()[0:C, :])
            nc.scalar.dma_start(out=wcat[:, U:2 * U], in_=w_d.ap()[C:2 * C, :])
            x_t = pool.tile([P, C], f32)
            nc.scalar.dma_start(out=x_t[:], in_=x_d.ap())

            # ---- constants (gpsimd, while DMAs are in flight) ----
            iota_f = pool.tile([P, N], f32)
            nc.gpsimd.iota(iota_f[:], pattern=[[1, N]], base=0,
                           channel_multiplier=0,
                           allow_small_or_imprecise_dtypes=True)
            xidx_f = pool.tile([P, 1], f32)
            nc.gpsimd.tensor_copy(xidx_f[:], xidx_i[:])

            # ---- u|v = x @ [W1|W2] (PE) ----
            mm = psum.tile([P, 2 * U], f32)    # [i, u | v]
            nc.tensor.matmul(mm[:], lhsT=xT[:], rhs=wcat[:], start=True,
                             stop=True)

            # ---- DVE: a_sel = adj[i, xidx_i] (gates the z1 chain), then
            # k = sum_j adj ----
            scr = pool.tile([P, N], f32)
            a_sel = pool.tile([P, 1], f32)
            nc.vector.scalar_tensor_tensor(
                out=scr[:], in0=iota_f[:], scalar=xidx_f[:, 0:1], in1=adj_t[:],
                op0=Alu.is_equal, op1=Alu.mult, accum_out=a_sel[:, 0:1])
            k = pool.tile([P, 1], f32)
            nc.vector.reduce_sum(k[:], adj_t[:], axis=AX)

            # s0 = any(b > 0) == (max(b) > 0), since z0 = relu(b)
            m0 = pool.tile([P, 1], f32)
            nc.vector.reduce_max(m0[:], bfull[:], axis=AX)
            s0 = pool.tile([P, 1], f32)
            nc.vector.tensor_scalar(out=s0[:], in0=m0[:], scalar1=0.0,
                                    scalar2=None, op0=Alu.is_gt)
            asm1 = pool.tile([P, 1], f32)
            nc.vector.tensor_scalar(out=asm1[:], in0=a_sel[:], scalar1=-1.0,
                                    scalar2=None, op0=Alu.add)

            # ---- gpsimd side: z0 = relu(b), k-derived scalars, xk ----
            z0 = pool.tile([P, U], f32)
            nc.gpsimd.tensor_scalar(out=z0[:], in0=bfull[:], scalar1=0.0,
                                    scalar2=None, op0=Alu.max)
            nk = pool.tile([P, 1], f32)
            nc.gpsimd.tensor_scalar(out=nk[:], in0=k[:], scalar1=-1.0,
                                    scalar2=float(N), op0=Alu.mult,
                                    op1=Alu.add)
            h0 = pool.tile([P, 1], f32)
            nc.gpsimd.tensor_scalar(out=h0[:], in0=k[:], scalar1=float(N),
                                    scalar2=None, op0=Alu.is_lt)
            h1 = pool.tile([P, 1], f32)
            nc.gpsimd.tensor_scalar(out=h1[:], in0=k[:], scalar1=0.0,
                                    scalar2=None, op0=Alu.is_gt)
            t2 = pool.tile([P, 1], f32)      # (N-k)*s0
            nc.gpsimd.tensor_mul(t2[:], nk[:], s0[:])
            na = pool.tile([P, 1], f32)      # k + t2
            nc.gpsimd.tensor_add(na[:], k[:], t2[:])
            # clamp t2 for 1/t2: t2=0 happens only when the s1=0 branch is
            # dead (k=N) or the reference itself divides by zero
            t2c = pool.tile([P, 1], f32)
            nc.gpsimd.tensor_scalar(out=t2c[:], in0=t2[:], scalar1=0.5,
                                    scalar2=None, op0=Alu.max)
            xk = pool.tile([P, C], f32)      # k*x
            nc.gpsimd.tensor_scalar_mul(xk[:], x_t[:], k[:, 0:1])
            z0h = pool.tile([P, U], f32)     # h0*z0
            nc.gpsimd.tensor_scalar_mul(z0h[:], z0[:], h0[:, 0:1])

            # ---- DVE main chain ----
            rnb = pool.tile([P, 1], f32)     # 1/t2        (s1 = 0 case)
            nc.vector.reciprocal(rnb[:], t2c[:])
            upb = pool.tile([P, U], f32)     # u + b
            nc.vector.tensor_add(upb[:], mm[:, 0:U], bfull[:])
            zz = pool.tile([P, U], f32)      # u + (a_sel-1)*v + b
            nc.vector.scalar_tensor_tensor(
                out=zz[:], in0=mm[:, U:2 * U], scalar=asm1[:, 0:1],
                in1=upb[:], op0=Alu.mult, op1=Alu.add)
            z1 = pool.tile([P, U], f32)
            z1sum = pool.tile([P, 1], f32)
            nc.vector.tensor_scalar(out=z1[:], in0=zz[:], scalar1=0.0,
                                    scalar2=None, op0=Alu.max, op1=Alu.add,
                                    accum_out=z1sum[:, 0:1])
            rna = pool.tile([P, 1], f32)     # 1/(k + t2)  (s1 = 1 case)
            nc.vector.reciprocal(rna[:], na[:])
            rnd = pool.tile([P, 1], f32)
            nc.vector.tensor_tensor(rnd[:], rna[:], rnb[:], op=Alu.subtract)
            s1 = pool.tile([P, 1], f32)
            nc.vector.tensor_scalar(out=s1[:], in0=z1sum[:], scalar1=0.0,
                                    scalar2=None, op0=Alu.is_gt)
            rn = pool.tile([P, 1], f32)      # 1/n = rnb + s1*(rna-rnb)
            nc.vector.scalar_tensor_tensor(
                out=rn[:], in0=rnd[:], scalar=s1[:, 0:1], in1=rnb[:],
                op0=Alu.mult, op1=Alu.add)

            out_t = pool.tile([P, OUTF], f32)
            # maxp = max(h1*z1, h0*z0): mult on gpsimd (parallel), max on DVE
            # (neither scalar_tensor_tensor nor TensorTensor-max is Pool-legal)
            z1h = pool.tile([P, U], f32)
            nc.gpsimd.tensor_scalar_mul(z1h[:], z1[:], h1[:, 0:1])
            nc.vector.tensor_scalar_mul(out_t[:, U:U + C], xk[:], rn[:, 0:1])
            nc.vector.tensor_scalar_mul(out_t[:, U + C:OUTF],
                                        out_t[:, U:U + C], asm1[:, 0:1])
            nc.vector.tensor_tensor(out_t[:, 0:U], z1h[:], z0h[:],
                                    op=Alu.max)
            nc.sync.dma_start(out=out_d.ap(), in_=out_t[:])

    nc.compile()
    return nc


def get_nc():
    if "nc" not in _CACHE:
        _CACHE["nc"] = _build_nc()
    return _CACHE["nc"]


def make_in_maps(inputs, adj_matrix, xidx, w, b):
    """Shard full inputs into per-core input maps (128 (b,i) rows per core)."""
    x_flat = np.ascontiguousarray(
        np.asarray(inputs, dtype=np.float32).reshape(B * N, C))
    adj_flat = np.ascontiguousarray(
        np.asarray(adj_matrix, dtype=np.float32).reshape(B * N, N))
    xidx_flat = np.ascontiguousarray(
        np.asarray(xidx, dtype=np.int32).reshape(B * N, 1))
    w_full = np.ascontiguousarray(np.asarray(w, dtype=np.float32)[0])
    b_full = np.ascontiguousarray(
        np.asarray(b, dtype=np.float32).reshape(1, U))

    in_maps = []
    for c in range(NCORES):
        rows = slice(c * P, (c + 1) * P)
        x_slab = x_flat[rows]
        in_maps.append({
            "adj": adj_flat[rows],
            "x": x_slab,
            "xT": np.ascontiguousarray(x_slab.T),
            "xidx": xidx_flat[rows],
            "w": w_full,
            "b": b_full,
        })
    return in_maps


def kernel(inputs, adj_matrix, xidx, w, b, _trace=False):
    from concourse.bass_utils import run_bass_kernel_spmd

    nc = get_nc()
    in_maps = make_in_maps(inputs, adj_matrix, xidx, w, b)
    res = run_bass_kernel_spmd(nc, in_maps, list(range(NCORES)),
                               trace=_trace)
    out = np.concatenate([res.results[c]["out"] for c in range(NCORES)],
                         axis=0)
    out = out.reshape(B, N, OUTF).astype(np.float32)
    if _trace:
        _CACHE["last_results"] = res
    return out


# revision 12
# speedup vs baseline: 1.3781x; 1.3781x over previous
"""Trainium2 Bass kernel for nn_EdgeConvolution (gnn_message_passing).

Math
----
Reference (B=2, N=512, C=128, U=128), adj binary {0,1}:
  masked[b,i,j,:]  = adj[b,i,j] * x[b,i,:]
  a_sel[b,i]       = adj[b,i, xidx[b,i]]
  edging[b,i,j,:]  = [ adj*x_i | adj*(a_sel - adj)*x_i ]
                   = adj[b,i,j] * [ x_i | (a_sel_i - 1)*x_i ]        (adj^2 = adj)
  out[b,i,j,:]     = relu(adj*(u_i + (a_sel_i-1)*v_i) + b),  u = x@W1, v = x@W2
So over j there are only two values per (b,i):
  z1_i = relu(u_i + (a_sel_i-1)*v_i + b)   (edges with adj=1, count k_i)
  z0   = relu(b)                            (edges with adj=0, count N-k_i)
  maxp_i   = max(1[k_i>0]*z1_i, 1[k_i<N]*z0)
  n_i      = k_i*1[any z1_i>0] + (N-k_i)*1[any z0>0]
  avgpool_i = [ k_i*x_i | k_i*(a_sel_i-1)*x_i ] / n_i
Per-core slab: 128 of the 1024 (b,i) rows; w/b replicated.

Implementation: raw Bass (no Tile) to minimize semaphore/barrier overhead.
Engines: SP ring DMAs (adj, xidx, b) + out; ACT ring DMAs (x|xT pack, w) +
per-partition-scale multiplies; PE: bias fold (ones x [b|0] accumulated into
x@[W1|W2]) and the b broadcast; DVE: reductions and the main chain; Pool:
iota/cast/[P,1] scalars. `n` is computed by selecting between the two
possible reciprocals so only one op depends on s1 = any(z1>0).
"""

import numpy as np

B, N, C, U = 2, 512, 128, 128
P = 128          # rows (b,i) per core == SBUF partitions
NCORES = 8
OUTF = U + 2 * C  # 384

_CACHE: dict = {}


def _build_nc():
    import concourse.bacc as bacc
    import concourse.bass as bass
    import concourse.mybir as mybir

    f32 = mybir.dt.float32
    i32 = mybir.dt.int32
    Alu = mybir.AluOpType
    AX = mybir.AxisListType.X
    Act = mybir.ActivationFunctionType

    nc = bacc.Bacc("TRN2", target_bir_lowering=False, debug=False,
                   num_devices=NCORES)

    adj_d = nc.dram_tensor("adj", [P, N], f32, kind="ExternalInput")
    xb_d = nc.dram_tensor("xboth", [P, 2 * C], f32, kind="ExternalInput")
    xidx_d = nc.dram_tensor("xidx", [P, 1], i32, kind="ExternalInput")
    w_d = nc.dram_tensor("w", [2 * C, U], f32, kind="ExternalInput")
    b_d = nc.dram_tensor("b", [1, U], f32, kind="ExternalInput")
    out_d = nc.dram_tensor("out", [P, OUTF], f32, kind="ExternalOutput")

    ctx_tensors = [
        ("adj_t", [P, N], f32), ("xb_t", [P, 2 * C], f32),
        ("wcat_t", [P, 2 * U], f32), ("xidx_t", [P, 1], i32),
        ("bpad_t", [1, 2 * U], f32), ("ones1", [1, P], f32),
        ("iota_f", [P, N], f32), ("xidx_f", [P, 1], f32),
        ("scr", [P, N], f32), ("zcol", [P, 1], f32), ("wscr", [P, 1], f32),
        ("a_sel", [P, 1], f32), ("k", [P, 1], f32), ("asm1", [P, 1], f32),
        ("t_sb", [P, U], f32), ("zz", [P, U], f32), ("z1", [P, U], f32),
        ("z1sum", [P, 1], f32), ("z0", [P, U], f32), ("z0sum", [P, 1], f32),
        ("s0", [P, 1], f32), ("nk", [P, 1], f32), ("h0", [P, 1], f32),
        ("h1", [P, 1], f32), ("t2", [P, 1], f32), ("na", [P, 1], f32),
        ("t2c", [P, 1], f32), ("rna", [P, 1], f32), ("rnb", [P, 1], f32),
        ("rnd", [P, 1], f32), ("s1", [P, 1], f32), ("rn", [P, 1], f32),
        ("xcat", [P, 2 * C], f32), ("z1h", [P, U], f32), ("z0h", [P, U], f32),
        ("out_t", [P, OUTF], f32),
    ]

    from contextlib import ExitStack
    with ExitStack() as ctx:
        t = {}
        for name, shape, dt in ctx_tensors:
            t[name] = ctx.enter_context(nc.sbuf_tensor(name, shape, dt))
        mm = ctx.enter_context(nc.psum_tensor("mm", [P, 2 * U], f32))
        bc = ctx.enter_context(nc.psum_tensor("bc", [P, U], f32))

        dadj = ctx.enter_context(nc.semaphore("dadj"))
        didx = ctx.enter_context(nc.semaphore("didx"))
        db = ctx.enter_context(nc.semaphore("db"))
        dxb = ctx.enter_context(nc.semaphore("dxb"))
        dwc = ctx.enter_context(nc.semaphore("dwc"))
        sini = ctx.enter_context(nc.semaphore("sini"))  # pool memsets done
        spe = ctx.enter_context(nc.semaphore("spe"))
        sdve = ctx.enter_context(nc.semaphore("sdve"))
        spool = ctx.enter_context(nc.semaphore("spool"))
        sact = ctx.enter_context(nc.semaphore("sact"))
        sfin = ctx.enter_context(nc.semaphore("sfin"))
        dout = ctx.enter_context(nc.semaphore("dout"))

        block = ctx.enter_context(nc.Block())

        ap = lambda h: h.ap()

        @block.gpsimd
        def _(pool):
            nc.gpsimd.memset(ap(t["ones1"]), 1.0)
            nc.gpsimd.memset(ap(t["bpad_t"]), 0.0)
            nc.gpsimd.memset(ap(t["zcol"]), 0.0)
            pool.drain().then_inc(sini, 1)
            nc.gpsimd.iota(ap(t["iota_f"]), pattern=[[1, N]], base=0,
                           channel_multiplier=0,
                           allow_small_or_imprecise_dtypes=True)
            pool.wait_ge(didx, 16)           # xidx landed
            nc.gpsimd.tensor_copy(ap(t["xidx_f"]), ap(t["xidx_t"]))
            pool.drain().then_inc(spool, 1)  # ->1: iota + xidx_f visible
            pool.wait_ge(sact, 1)            # z0sum ready
            nc.gpsimd.tensor_scalar(out=ap(t["s0"]), in0=ap(t["z0sum"]),
                                    scalar1=0.0, scalar2=None, op0=Alu.is_gt)
            pool.wait_ge(sdve, 1)            # k ready
            nc.gpsimd.tensor_scalar(out=ap(t["nk"]), in0=ap(t["k"]),
                                    scalar1=-1.0, scalar2=float(N),
                                    op0=Alu.mult, op1=Alu.add)
            nc.gpsimd.tensor_scalar(out=ap(t["h0"]), in0=ap(t["k"]),
                                    scalar1=float(N), scalar2=None,
                                    op0=Alu.is_lt)
            nc.gpsimd.tensor_scalar(out=ap(t["h1"]), in0=ap(t["k"]),
                                    scalar1=0.0, scalar2=None, op0=Alu.is_gt)
            pool.drain().then_inc(spool, 1)  # ->2: s0, nk, h0, h1 visible
            nc.gpsimd.tensor_mul(ap(t["t2"]), ap(t["nk"]), ap(t["s0"]))
            pool.drain()                     # t2 -> na/t2c (same engine)
            nc.gpsimd.tensor_add(ap(t["na"]), ap(t["k"]), ap(t["t2"]))
            nc.gpsimd.tensor_scalar(out=ap(t["t2c"]), in0=ap(t["t2"]),
                                    scalar1=0.5, scalar2=None, op0=Alu.max)
            pool.drain().then_inc(spool, 1)  # ->3: t2, na, t2c visible

        @block.sync
        def _(sync):
            sync.dma_start(ap(t["adj_t"]), adj_d.ap()).then_inc(dadj, 16)
            sync.dma_start(ap(t["xidx_t"]), xidx_d.ap()).then_inc(didx, 16)
            sync.wait_ge(sini, 1)
            sync.dma_start(t["bpad_t"].ap()[0:1, 0:U],
                           b_d.ap()).then_inc(db, 16)
            sync.wait_ge(sfin, 2)
            sync.dma_start(out_d.ap(), ap(t["out_t"])).then_inc(dout, 16)
            sync.wait_ge(dout, 16)

        @block.scalar
        def _(act):
            act.dma_start(ap(t["xb_t"]), xb_d.ap()).then_inc(dxb, 16)
            act.dma_start(
                t["wcat_t"].ap().rearrange("p (s u) -> p s u", s=2),
                w_d.ap().rearrange("(s c) u -> c s u", s=2),
            ).then_inc(dwc, 16)
            act.wait_ge(sini, 1)
            # warm the activation table off the critical path
            nc.scalar.activation(out=ap(t["wscr"]), in_=ap(t["zcol"]),
                                 func=Act.Relu, bias=t["zcol"].ap()[:, 0:1])
            act.wait_ge(spe, 1)              # bc = ones x b broadcast done
            nc.scalar.activation(out=ap(t["z0"]), in_=bc.ap(), func=Act.Relu,
                                 bias=t["zcol"].ap()[:, 0:1],
                                 accum_out=t["z0sum"].ap()[:, 0:1]
                                 ).then_inc(sact, 1)                    # ->1
            act.wait_ge(dxb, 16)             # xb landed
            act.wait_ge(sdve, 1)             # k
            nc.scalar.activation(out=t["xcat"].ap()[:, 0:C],
                                 in_=t["xb_t"].ap()[:, 0:C], func=Act.Copy,
                                 scale=t["k"].ap()[:, 0:1])
            act.drain()                      # xk -> xka; z0 -> z0h
            act.wait_ge(sdve, 3)             # asm1
            nc.scalar.activation(out=t["xcat"].ap()[:, C:2 * C],
                                 in_=t["xcat"].ap()[:, 0:C], func=Act.Copy,
                                 scale=t["asm1"].ap()[:, 0:1])
            act.wait_ge(spool, 2)            # h0, h1
            nc.scalar.activation(out=ap(t["z0h"]), in_=ap(t["z0"]),
                                 func=Act.Copy, scale=t["h0"].ap()[:, 0:1])
            act.wait_ge(sdve, 4)             # z1
            nc.scalar.activation(out=ap(t["z1h"]), in_=ap(t["z1"]),
                                 func=Act.Copy, scale=t["h1"].ap()[:, 0:1])
            act.drain().then_inc(sact, 1)    # ->2: z0h + z1h visible
            act.wait_ge(sdve, 5)             # rn
            nc.scalar.activation(out=t["out_t"].ap()[:, U:OUTF],
                                 in_=ap(t["xcat"]), func=Act.Copy,
                                 scale=t["rn"].ap()[:, 0:1])
            act.drain().then_inc(sfin, 1)

        @block.tensor
        def _(pe):
            pe.wait_ge(db, 16)               # b landed (ones1/bpad via sini)
            nc.tensor.matmul(bc.ap(), lhsT=t["ones1"].ap(),
                             rhs=t["bpad_t"].ap()[0:1, 0:U],
                             start=True, stop=True).then_inc(spe, 1)
            nc.tensor.matmul(mm.ap(), lhsT=t["ones1"].ap(),
                             rhs=t["bpad_t"].ap(), start=True, stop=False)
            pe.wait_ge(dxb, 16)              # xb landed
            pe.wait_ge(dwc, 16)              # wcat landed
            nc.tensor.matmul(mm.ap(), lhsT=t["xb_t"].ap()[:, C:2 * C],
                             rhs=t["wcat_t"].ap(), start=False,
                             stop=True).then_inc(spe, 1)                # ->2

        @block.vector
        def _(dve):
            dve.wait_ge(dadj, 16)            # adj landed
            nc.vector.reduce_sum(ap(t["k"]), ap(t["adj_t"]), axis=AX)
            dve.drain().then_inc(sdve, 1)    # ->1: k visible
            dve.wait_ge(spool, 1)            # iota + xidx_f
            nc.vector.scalar_tensor_tensor(
                out=ap(t["scr"]), in0=ap(t["iota_f"]),
                scalar=t["xidx_f"].ap()[:, 0:1], in1=ap(t["adj_t"]),
                op0=Alu.is_equal, op1=Alu.mult,
                accum_out=t["a_sel"].ap()[:, 0:1]).then_inc(sdve, 1)    # ->2
            dve.wait_ge(sdve, 2)             # a_sel accum lands async
            nc.vector.tensor_scalar(out=ap(t["asm1"]), in0=ap(t["a_sel"]),
                                    scalar1=-1.0, scalar2=None, op0=Alu.add)
            dve.drain().then_inc(sdve, 1)    # ->3: asm1 visible
            dve.wait_ge(spe, 2)              # mm = [u+b | v]
            nc.vector.tensor_scalar(out=ap(t["t_sb"]),
                                    in0=mm.ap()[:, U:2 * U],
                                    scalar1=t["asm1"].ap()[:, 0:1],
                                    scalar2=None, op0=Alu.mult)
            dve.drain()
            nc.vector.tensor_add(ap(t["zz"]), ap(t["t_sb"]), mm.ap()[:, 0:U])
            dve.drain()
            nc.vector.tensor_scalar(out=ap(t["z1"]), in0=ap(t["zz"]),
                                    scalar1=0.0, scalar2=None, op0=Alu.max,
                                    op1=Alu.add,
                                    accum_out=t["z1sum"].ap()[:, 0:1]
                                    ).then_inc(sdve, 1)                 # ->4
            dve.wait_ge(spool, 3)            # t2, na, t2c
            nc.vector.reciprocal(ap(t["rna"]), ap(t["na"]))
            nc.vector.reciprocal(ap(t["rnb"]), ap(t["t2c"]))
            dve.drain()                      # rna/rnb -> rnd
            nc.vector.tensor_tensor(ap(t["rnd"]), ap(t["rna"]), ap(t["rnb"]),
                                    op=Alu.subtract)
            dve.wait_ge(sdve, 4)             # z1sum accum lands async
            nc.vector.tensor_scalar(out=ap(t["s1"]), in0=ap(t["z1sum"]),
                                    scalar1=0.0, scalar2=None, op0=Alu.is_gt)
            dve.drain()                      # rnd, s1 -> rn
            nc.vector.scalar_tensor_tensor(
                out=ap(t["rn"]), in0=ap(t["rnd"]),
                scalar=t["s1"].ap()[:, 0:1], in1=ap(t["rnb"]),
                op0=Alu.mult, op1=Alu.add)
            dve.drain().then_inc(sdve, 1)    # ->5: rn visible
            dve.wait_ge(sact, 2)             # z0h + z1h
            nc.vector.tensor_tensor(t["out_t"].ap()[:, 0:U], ap(t["z1h"]),
                                    ap(t["z0h"]), op=Alu.max)
            dve.drain().then_inc(sfin, 1)

    nc.compile()
    return nc


def get_nc():
    if "nc" not in _CACHE:
        _CACHE["nc"] = _build_nc()
    return _CACHE["nc"]


def make_in_maps(inputs, adj_matrix, xidx, w, b):
    """Shard full inputs into per-core input maps (128 (b,i) rows per core)."""
    x_flat = np.asarray(inputs, dtype=np.float32).reshape(B * N, C)
    adj_flat = np.ascontiguousarray(
        np.asarray(adj_matrix, dtype=np.float32).reshape(B * N, N))
    xidx_flat = np.ascontiguousarray(
        np.asarray(xidx, dtype=np.int32).reshape(B * N, 1))
    w_full = np.ascontiguousarray(np.asarray(w, dtype=np.float32)[0])
    b_full = np.ascontiguousarray(
        np.asarray(b, dtype=np.float32).reshape(1, U))

    in_maps = []
    for c in range(NCORES):
        rows = slice(c * P, (c + 1) * P)
        x_slab = x_flat[rows]
        in_maps.append({
            "adj": adj_flat[rows],
            "xboth": np.ascontiguousarray(
                np.concatenate([x_slab, x_slab.T], axis=1)),
            "xidx": xidx_flat[rows],
            "w": w_full,
            "b": b_full,
        })
    return in_maps


def kernel(inputs, adj_matrix, xidx, w, b, _trace=False):
    from concourse.bass_utils import run_bass_kernel_spmd

    nc = get_nc()
    in_maps = make_in_maps(inputs, adj_matrix, xidx, w, b)
    res = run_bass_kernel_spmd(nc, in_maps, list(range(NCORES)),
                               trace=_trace)
    out = np.concatenate([res.results[c]["out"] for c in range(NCORES)],
                         axis=0)
    out = out.reshape(B, N, OUTF).astype(np.float32)
    if _trace:
        _CACHE["last_results"] = res
    return out


# revision 14
# speedup vs baseline: 1.4434x; 1.0474x over previous
"""Trainium2 Bass kernel for nn_EdgeConvolution (gnn_message_passing).

Math
----
Reference (B=2, N=512, C=128, U=128), adj binary {0,1}:
  masked[b,i,j,:]  = adj[b,i,j] * x[b,i,:]
  a_sel[b,i]       = adj[b,i, xidx[b,i]]
  edging[b,i,j,:]  = [ adj*x_i | adj*(a_sel - adj)*x_i ]
                   = adj[b,i,j] * [ x_i | (a_sel_i - 1)*x_i ]        (adj^2 = adj)
  out[b,i,j,:]     = relu(adj*(u_i + (a_sel_i-1)*v_i) + b),  u = x@W1, v = x@W2
So over j there are only two values per (b,i):
  z1_i = relu(u_i + (a_sel_i-1)*v_i + b)   (edges with adj=1, count k_i)
  z0   = relu(b)                            (edges with adj=0, count N-k_i)
  maxp_i   = max(1[k_i>0]*z1_i, 1[k_i<N]*z0)
  n_i      = k_i*1[any z1_i>0] + (N-k_i)*1[any z0>0]
  avgpool_i = [ k_i*x_i | k_i*(a_sel_i-1)*x_i ] / n_i
Per-core slab: 128 of the 1024 (b,i) rows; w/b replicated.

Implementation: raw Bass (no Tile) to minimize semaphore/barrier overhead.
Engines: SP ring DMAs (adj, xidx, b) + out; ACT ring DMAs (x|xT pack, w) +
per-partition-scale multiplies; PE: bias fold (ones x [b|0] accumulated into
x@[W1|W2]) and the b broadcast; DVE: reductions and the main chain; Pool:
iota/cast/[P,1] scalars. `n` is computed by selecting between the two
possible reciprocals so only one op depends on s1 = any(z1>0).
"""

import numpy as np

B, N, C, U = 2, 512, 128, 128
P = 128          # rows (b,i) per core == SBUF partitions
NCORES = 8
OUTF = U + 2 * C  # 384

_CACHE: dict = {}


def _build_nc():
    import concourse.bacc as bacc
    import concourse.bass as bass
    import concourse.mybir as mybir

    f32 = mybir.dt.float32
    i32 = mybir.dt.int32
    Alu = mybir.AluOpType
    AX = mybir.AxisListType.X
    Act = mybir.ActivationFunctionType

    nc = bacc.Bacc("TRN2", target_bir_lowering=False, debug=False,
                   num_devices=NCORES)

    adj_d = nc.dram_tensor("adj", [P, N], f32, kind="ExternalInput")
    xb_d = nc.dram_tensor("xboth", [P, 2 * C], f32, kind="ExternalInput")
    xidx_d = nc.dram_tensor("xidx", [P, 1], i32, kind="ExternalInput")
    w_d = nc.dram_tensor("w", [2 * C, U], f32, kind="ExternalInput")
    b_d = nc.dram_tensor("b", [1, U], f32, kind="ExternalInput")
    out_d = nc.dram_tensor("out", [P, OUTF], f32, kind="ExternalOutput")

    ctx_tensors = [
        ("adj_t", [P, N], f32), ("xb_t", [P, 2 * C], f32),
        ("wcat_t", [P, 2 * U], f32), ("xidx_t", [P, 1], i32),
        ("bpad_t", [1, 2 * U], f32), ("ones1", [1, P], f32),
        ("iota_f", [P, N], f32), ("xidx_f", [P, 1], f32),
        ("scr", [P, N], f32), ("zcol", [P, 1], f32), ("wscr", [P, 1], f32),
        ("a_sel", [P, 1], f32), ("k", [P, 1], f32), ("asm1", [P, 1], f32),
        ("t_sb", [P, U], f32), ("zz", [P, U], f32), ("z1", [P, U], f32),
        ("z1sum", [P, 1], f32), ("z0", [P, U], f32), ("z0sum", [P, 1], f32),
        ("s0", [P, 1], f32), ("nk", [P, 1], f32), ("h0", [P, 1], f32),
        ("h1", [P, 1], f32), ("t2", [P, 1], f32), ("na", [P, 1], f32),
        ("t2c", [P, 1], f32), ("rna", [P, 1], f32), ("rnb", [P, 1], f32),
        ("rnd", [P, 1], f32), ("s1", [P, 1], f32), ("rn", [P, 1], f32),
        ("xcat", [P, 2 * C], f32), ("z1h", [P, U], f32), ("z0h", [P, U], f32),
        ("out_t", [P, OUTF], f32),
    ]

    from contextlib import ExitStack
    with ExitStack() as ctx:
        t = {}
        for name, shape, dt in ctx_tensors:
            t[name] = ctx.enter_context(nc.sbuf_tensor(name, shape, dt))
        mm = ctx.enter_context(nc.psum_tensor("mm", [P, 2 * U], f32))

        dadj = ctx.enter_context(nc.semaphore("dadj"))
        didx = ctx.enter_context(nc.semaphore("didx"))
        db = ctx.enter_context(nc.semaphore("db"))
        dxb = ctx.enter_context(nc.semaphore("dxb"))
        dwc = ctx.enter_context(nc.semaphore("dwc"))
        sini = ctx.enter_context(nc.semaphore("sini"))
        spe = ctx.enter_context(nc.semaphore("spe"))
        sdve = ctx.enter_context(nc.semaphore("sdve"))
        spool = ctx.enter_context(nc.semaphore("spool"))
        sact = ctx.enter_context(nc.semaphore("sact"))
        sz0 = ctx.enter_context(nc.semaphore("sz0"))
        sfin = ctx.enter_context(nc.semaphore("sfin"))
        dout = ctx.enter_context(nc.semaphore("dout"))

        block = ctx.enter_context(nc.Block())

        ap = lambda h: h.ap()

        # Self-waits use all-incs-so-far thresholds: completions on one
        # engine can retire out of order, so `>= total` is the only
        # order-independent guarantee that a specific producer finished.

        @block.gpsimd
        def _(pool):
            nc.gpsimd.memset(ap(t["ones1"]), 1.0)
            nc.gpsimd.memset(ap(t["bpad_t"]), 0.0)
            nc.gpsimd.memset(ap(t["zcol"]), 0.0)
            pool.drain().then_inc(sini, 1)
            nc.gpsimd.iota(ap(t["iota_f"]), pattern=[[1, N]], base=0,
                           channel_multiplier=0,
                           allow_small_or_imprecise_dtypes=True
                           ).then_inc(spool, 1)                       # ->1
            pool.wait_ge(didx, 16)
            nc.gpsimd.tensor_copy(ap(t["xidx_f"]),
                                  ap(t["xidx_t"])).then_inc(spool, 1)  # ->2
            pool.wait_ge(sz0, 1)             # z0sum ready
            nc.gpsimd.tensor_scalar(out=ap(t["s0"]), in0=ap(t["z0sum"]),
                                    scalar1=0.0, scalar2=None,
                                    op0=Alu.is_gt).then_inc(spool, 1)  # ->3
            pool.wait_ge(sdve, 1)            # k ready
            nc.gpsimd.tensor_scalar(out=ap(t["nk"]), in0=ap(t["k"]),
                                    scalar1=-1.0, scalar2=float(N),
                                    op0=Alu.mult,
                                    op1=Alu.add).then_inc(spool, 1)    # ->4
            nc.gpsimd.tensor_scalar(out=ap(t["h0"]), in0=ap(t["k"]),
                                    scalar1=float(N), scalar2=None,
                                    op0=Alu.is_lt).then_inc(spool, 1)  # ->5
            nc.gpsimd.tensor_scalar(out=ap(t["h1"]), in0=ap(t["k"]),
                                    scalar1=0.0, scalar2=None,
                                    op0=Alu.is_gt).then_inc(spool, 1)  # ->6
            pool.wait_ge(spool, 6)           # s0 + nk visible (all 6)
            nc.gpsimd.tensor_mul(ap(t["t2"]), ap(t["nk"]),
                                 ap(t["s0"])).then_inc(spool, 1)       # ->7
            pool.wait_ge(spool, 7)           # t2 visible
            nc.gpsimd.tensor_add(ap(t["na"]), ap(t["k"]),
                                 ap(t["t2"])).then_inc(spool, 1)       # ->8
            nc.gpsimd.tensor_scalar(out=ap(t["t2c"]), in0=ap(t["t2"]),
                                    scalar1=0.5, scalar2=None,
                                    op0=Alu.max).then_inc(spool, 1)    # ->9

        @block.sync
        def _(sync):
            sync.dma_start(ap(t["adj_t"]), adj_d.ap()).then_inc(dadj, 16)
            sync.wait_ge(sini, 1)
            sync.dma_start(t["bpad_t"].ap()[0:1, 0:U],
                           b_d.ap()).then_inc(db, 16)
            sync.dma_start(ap(t["xidx_t"]), xidx_d.ap()).then_inc(didx, 16)
            sync.wait_ge(sfin, 2)
            sync.dma_start(out_d.ap(), ap(t["out_t"])).then_inc(dout, 16)
            sync.wait_ge(dout, 16)

        @block.scalar
        def _(act):
            act.dma_start(ap(t["xb_t"]), xb_d.ap()).then_inc(dxb, 16)
            act.dma_start(
                t["wcat_t"].ap().rearrange("p (s u) -> p s u", s=2),
                w_d.ap().rearrange("(s c) u -> c s u", s=2),
            ).then_inc(dwc, 16)
            act.wait_ge(sini, 1)
            # warm the activation table off the critical path
            nc.scalar.activation(out=ap(t["wscr"]), in_=ap(t["zcol"]),
                                 func=Act.Relu, bias=t["zcol"].ap()[:, 0:1])
            act.wait_ge(spe, 1)              # bias matmul: mm = [b|0] bcast
            nc.scalar.activation(out=ap(t["z0"]), in_=mm.ap()[:, 0:U],
                                 func=Act.Relu, bias=t["zcol"].ap()[:, 0:1],
                                 accum_out=t["z0sum"].ap()[:, 0:1]
                                 ).then_inc(sz0, 1)       # frees PE main mm
            act.wait_ge(dxb, 16)
            act.wait_ge(sdve, 1)             # k
            nc.scalar.activation(out=t["xcat"].ap()[:, 0:C],
                                 in_=t["xb_t"].ap()[:, 0:C], func=Act.Copy,
                                 scale=t["k"].ap()[:, 0:1]
                                 ).then_inc(sact, 1)                   # ->1
            act.wait_ge(sdve, 3)             # asm1
            act.wait_ge(sact, 1)             # xk visible (self)
            nc.scalar.activation(out=t["xcat"].ap()[:, C:2 * C],
                                 in_=t["xcat"].ap()[:, 0:C], func=Act.Copy,
                                 scale=t["asm1"].ap()[:, 0:1]
                                 ).then_inc(sact, 1)                   # ->2
            act.wait_ge(spool, 6)            # h0, h1
            act.wait_ge(sact, 2)             # z0 visible (z0acc done, all 2)
            nc.scalar.activation(out=ap(t["z0h"]), in_=ap(t["z0"]),
                                 func=Act.Copy, scale=t["h0"].ap()[:, 0:1]
                                 ).then_inc(sact, 1)                   # ->3
            act.wait_ge(sdve, 8)             # z1
            nc.scalar.activation(out=ap(t["z1h"]), in_=ap(t["z1"]),
                                 func=Act.Copy, scale=t["h1"].ap()[:, 0:1]
                                 ).then_inc(sact, 1)                   # ->4
            act.wait_ge(sdve, 11)            # rn
            act.wait_ge(sact, 4)             # xcat fully visible
            nc.scalar.activation(out=t["out_t"].ap()[:, U:OUTF],
                                 in_=ap(t["xcat"]), func=Act.Copy,
                                 scale=t["rn"].ap()[:, 0:1]
                                 ).then_inc(sfin, 1)

        @block.tensor
        def _(pe):
            pe.wait_ge(db, 16)               # b landed (ones1/bpad via sini)
            nc.tensor.matmul(mm.ap(), lhsT=t["ones1"].ap(),
                             rhs=t["bpad_t"].ap(), start=True,
                             stop=True).then_inc(spe, 1)
            pe.wait_ge(sz0, 1)               # ACT read the [b|0] broadcast
            pe.wait_ge(dxb, 16)
            pe.wait_ge(dwc, 16)
            nc.tensor.matmul(mm.ap(), lhsT=t["xb_t"].ap()[:, C:2 * C],
                             rhs=t["wcat_t"].ap(), start=False,
                             stop=True,
                             skip_group_check=True).then_inc(spe, 1)   # ->2

        @block.vector
        def _(dve):
            dve.wait_ge(dadj, 16)
            nc.vector.reduce_sum(ap(t["k"]), ap(t["adj_t"]),
                                 axis=AX).then_inc(sdve, 1)            # ->1
            dve.wait_ge(spool, 2)            # iota + xidx_f
            nc.vector.scalar_tensor_tensor(
                out=ap(t["scr"]), in0=ap(t["iota_f"]),
                scalar=t["xidx_f"].ap()[:, 0:1], in1=ap(t["adj_t"]),
                op0=Alu.is_equal, op1=Alu.mult,
                accum_out=t["a_sel"].ap()[:, 0:1]).then_inc(sdve, 1)   # ->2
            dve.wait_ge(sdve, 2)             # a_sel accum lands async
            nc.vector.tensor_scalar(out=ap(t["asm1"]), in0=ap(t["a_sel"]),
                                    scalar1=-1.0, scalar2=None,
                                    op0=Alu.add).then_inc(sdve, 1)     # ->3
            dve.wait_ge(spe, 2)              # mm = [u+b | v]
            dve.wait_ge(sdve, 3)             # asm1 visible
            nc.vector.tensor_scalar(out=ap(t["t_sb"]),
                                    in0=mm.ap()[:, U:2 * U],
                                    scalar1=t["asm1"].ap()[:, 0:1],
                                    scalar2=None,
                                    op0=Alu.mult).then_inc(sdve, 1)    # ->4
            dve.wait_ge(spool, 9)            # na, t2c
            nc.vector.reciprocal(ap(t["rna"]),
                                 ap(t["na"])).then_inc(sdve, 1)        # ->5
            nc.vector.reciprocal(ap(t["rnb"]),
                                 ap(t["t2c"])).then_inc(sdve, 1)       # ->6
            dve.wait_ge(sdve, 4)             # t_sb visible
            nc.vector.tensor_add(ap(t["zz"]), ap(t["t_sb"]),
                                 mm.ap()[:, 0:U]).then_inc(sdve, 1)    # ->7
            dve.wait_ge(sdve, 7)             # zz visible
            nc.vector.tensor_scalar(out=ap(t["z1"]), in0=ap(t["zz"]),
                                    scalar1=0.0, scalar2=None, op0=Alu.max,
                                    op1=Alu.add,
                                    accum_out=t["z1sum"].ap()[:, 0:1]
                                    ).then_inc(sdve, 1)                # ->8
            dve.wait_ge(sdve, 6)             # rna + rnb visible
            nc.vector.tensor_tensor(ap(t["rnd"]), ap(t["rna"]), ap(t["rnb"]),
                                    op=Alu.subtract).then_inc(sdve, 1)  # ->9
            dve.wait_ge(sdve, 8)             # z1sum accum landed
            nc.vector.tensor_scalar(out=ap(t["s1"]), in0=ap(t["z1sum"]),
                                    scalar1=0.0, scalar2=None,
                                    op0=Alu.is_gt).then_inc(sdve, 1)   # ->10
            dve.wait_ge(sdve, 10)            # rnd + s1 visible
            nc.vector.scalar_tensor_tensor(
                out=ap(t["rn"]), in0=ap(t["rnd"]),
                scalar=t["s1"].ap()[:, 0:1], in1=ap(t["rnb"]),
                op0=Alu.mult, op1=Alu.add).then_inc(sdve, 1)           # ->11
            dve.wait_ge(sact, 4)             # z0h + z1h
            nc.vector.tensor_tensor(t["out_t"].ap()[:, 0:U], ap(t["z1h"]),
                                    ap(t["z0h"]), op=Alu.max
                                    ).then_inc(sfin, 1)

    nc.compile()
    return nc


def get_nc():
    if "nc" not in _CACHE:
        _CACHE["nc"] = _build_nc()
    return _CACHE["nc"]


def make_in_maps(inputs, adj_matrix, xidx, w, b):
    """Shard full inputs into per-core input maps (128 (b,i) rows per core)."""
    x_flat = np.asarray(inputs, dtype=np.float32).reshape(B * N, C)
    adj_flat = np.ascontiguousarray(
        np.asarray(adj_matrix, dtype=np.float32).reshape(B * N, N))
    xidx_flat = np.ascontiguousarray(
        np.asarray(xidx, dtype=np.int32).reshape(B * N, 1))
    w_full = np.ascontiguousarray(np.asarray(w, dtype=np.float32)[0])
    b_full = np.ascontiguousarray(
        np.asarray(b, dtype=np.float32).reshape(1, U))

    in_maps = []
    for c in range(NCORES):
        rows = slice(c * P, (c + 1) * P)
        x_slab = x_flat[rows]
        in_maps.append({
            "adj": adj_flat[rows],
            "xboth": np.ascontiguousarray(
                np.concatenate([x_slab, x_slab.T], axis=1)),
            "xidx": xidx_flat[rows],
            "w": w_full,
            "b": b_full,
        })
    return in_maps


def kernel(inputs, adj_matrix, xidx, w, b, _trace=False):
    from concourse.bass_utils import run_bass_kernel_spmd

    nc = get_nc()
    in_maps = make_in_maps(inputs, adj_matrix, xidx, w, b)
    res = run_bass_kernel_spmd(nc, in_maps, list(range(NCORES)),
                               trace=_trace)
    out = np.concatenate([res.results[c]["out"] for c in range(NCORES)],
                         axis=0)
    out = out.reshape(B, N, OUTF).astype(np.float32)
    if _trace:
        _CACHE["last_results"] = res
    return out


# revision 16
# speedup vs baseline: 1.4656x; 1.0154x over previous
"""Trainium2 Bass kernel for nn_EdgeConvolution (gnn_message_passing).

Math
----
Reference (B=2, N=512, C=128, U=128), adj binary {0,1}:
  masked[b,i,j,:]  = adj[b,i,j] * x[b,i,:]
  a_sel[b,i]       = adj[b,i, xidx[b,i]]
  edging[b,i,j,:]  = [ adj*x_i | adj*(a_sel - adj)*x_i ]
                   = adj[b,i,j] * [ x_i | (a_sel_i - 1)*x_i ]        (adj^2 = adj)
  out[b,i,j,:]     = relu(adj*(u_i + (a_sel_i-1)*v_i) + b),  u = x@W1, v = x@W2
So over j there are only two values per (b,i):
  z1_i = relu(u_i + (a_sel_i-1)*v_i + b)   (edges with adj=1, count k_i)
  z0   = relu(b)                            (edges with adj=0, count N-k_i)
  maxp_i   = max(1[k_i>0]*z1_i, 1[k_i<N]*z0)
  n_i      = k_i*1[any z1_i>0] + (N-k_i)*1[any z0>0]
  avgpool_i = [ k_i*x_i | k_i*(a_sel_i-1)*x_i ] / n_i
Per-core slab: 128 of the 1024 (b,i) rows; w/b replicated.

Implementation: raw Bass (no Tile) to minimize semaphore/barrier overhead.
Engines: SP ring DMAs (adj, xidx, b) + out; ACT ring DMAs (x|xT pack, w) +
per-partition-scale multiplies; PE: bias fold (ones x [b|0] accumulated into
x@[W1|W2]) and the b broadcast; DVE: reductions and the main chain; Pool:
iota/cast/[P,1] scalars. `n` is computed by selecting between the two
possible reciprocals so only one op depends on s1 = any(z1>0).
"""

import numpy as np

B, N, C, U = 2, 512, 128, 128
P = 128          # rows (b,i) per core == SBUF partitions
NCORES = 8
OUTF = U + 2 * C  # 384

_CACHE: dict = {}


def _build_nc():
    import concourse.bacc as bacc
    import concourse.bass as bass
    import concourse.mybir as mybir

    f32 = mybir.dt.float32
    i32 = mybir.dt.int32
    Alu = mybir.AluOpType
    AX = mybir.AxisListType.X
    Act = mybir.ActivationFunctionType

    nc = bacc.Bacc("TRN2", target_bir_lowering=False, debug=False,
                   num_devices=NCORES)

    adj_d = nc.dram_tensor("adj", [P, N], f32, kind="ExternalInput")
    xb_d = nc.dram_tensor("xboth", [P, 2 * C], f32, kind="ExternalInput")
    xidx_d = nc.dram_tensor("xidx", [P, 1], i32, kind="ExternalInput")
    w_d = nc.dram_tensor("w", [2 * C, U], f32, kind="ExternalInput")
    b_d = nc.dram_tensor("b", [1, U], f32, kind="ExternalInput")
    out_d = nc.dram_tensor("out", [P, OUTF], f32, kind="ExternalOutput")

    ctx_tensors = [
        ("adj_t", [P, N], f32), ("xb_t", [P, 2 * C], f32),
        ("wcat_t", [P, 2 * U], f32), ("xidx_t", [P, 1], i32),
        ("brow_t", [1, U], f32), ("ones1", [1, P], f32),
        ("iota_f", [P, N], f32), ("xidx_f", [P, 1], f32),
        ("scr", [P, N], f32), ("zcol", [P, 1], f32), ("wscr", [P, 1], f32),
        ("a_sel", [P, 1], f32), ("k", [P, 1], f32), ("asm1", [P, 1], f32),
        ("t_sb", [P, U], f32), ("zz", [P, U], f32), ("zzb", [P, U], f32),
        ("z1", [P, U], f32),
        ("z1sum", [P, 1], f32), ("z0", [P, U], f32), ("z0sum", [P, 1], f32),
        ("s0", [P, 1], f32), ("nk", [P, 1], f32), ("h0", [P, 1], f32),
        ("h1", [P, 1], f32), ("t2", [P, 1], f32),
        ("s1", [P, 1], f32), ("nn", [P, 1], f32), ("rn", [P, 1], f32),
        ("xcat", [P, 2 * C], f32), ("z0h", [P, U], f32),
        ("out_t", [P, OUTF], f32),
    ]

    from contextlib import ExitStack
    with ExitStack() as ctx:
        t = {}
        for name, shape, dt in ctx_tensors:
            t[name] = ctx.enter_context(nc.sbuf_tensor(name, shape, dt))
        mm = ctx.enter_context(nc.psum_tensor("mm", [P, 2 * U], f32))
        bc = ctx.enter_context(nc.psum_tensor("bc", [P, U], f32))

        dadj = ctx.enter_context(nc.semaphore("dadj"))
        didx = ctx.enter_context(nc.semaphore("didx"))
        db = ctx.enter_context(nc.semaphore("db"))
        dxb = ctx.enter_context(nc.semaphore("dxb"))
        dwc = ctx.enter_context(nc.semaphore("dwc"))
        sini = ctx.enter_context(nc.semaphore("sini"))
        spe = ctx.enter_context(nc.semaphore("spe"))
        sdve = ctx.enter_context(nc.semaphore("sdve"))
        spool = ctx.enter_context(nc.semaphore("spool"))
        sact = ctx.enter_context(nc.semaphore("sact"))
        sz0 = ctx.enter_context(nc.semaphore("sz0"))
        sfin = ctx.enter_context(nc.semaphore("sfin"))
        dout = ctx.enter_context(nc.semaphore("dout"))

        block = ctx.enter_context(nc.Block())

        ap = lambda h: h.ap()

        # Self-waits use all-incs-so-far thresholds: completions on one
        # engine can retire out of order, so `>= total` is the only
        # order-independent guarantee that a specific producer finished.

        @block.gpsimd
        def _(pool):
            nc.gpsimd.memset(ap(t["ones1"]), 1.0)
            nc.gpsimd.memset(ap(t["zcol"]), 0.0)
            pool.drain().then_inc(sini, 1)
            nc.gpsimd.iota(ap(t["iota_f"]), pattern=[[1, N]], base=0,
                           channel_multiplier=0,
                           allow_small_or_imprecise_dtypes=True
                           ).then_inc(spool, 1)                        # ->1
            pool.wait_ge(didx, 16)
            nc.gpsimd.tensor_copy(ap(t["xidx_f"]),
                                  ap(t["xidx_t"])).then_inc(spool, 1)  # ->2
            pool.wait_ge(sdve, 1)            # k ready
            nc.gpsimd.tensor_scalar(out=ap(t["nk"]), in0=ap(t["k"]),
                                    scalar1=-1.0, scalar2=float(N),
                                    op0=Alu.mult,
                                    op1=Alu.add).then_inc(spool, 1)    # ->3
            nc.gpsimd.tensor_scalar(out=ap(t["h0"]), in0=ap(t["k"]),
                                    scalar1=float(N), scalar2=None,
                                    op0=Alu.is_lt).then_inc(spool, 1)  # ->4
            nc.gpsimd.tensor_scalar(out=ap(t["h1"]), in0=ap(t["k"]),
                                    scalar1=0.0, scalar2=None,
                                    op0=Alu.is_gt).then_inc(spool, 1)  # ->5
            pool.wait_ge(sz0, 1)             # z0sum ready
            nc.gpsimd.tensor_scalar(out=ap(t["s0"]), in0=ap(t["z0sum"]),
                                    scalar1=0.0, scalar2=None,
                                    op0=Alu.is_gt).then_inc(spool, 1)  # ->6
            pool.wait_ge(spool, 6)           # nk + s0 visible (all 6)
            nc.gpsimd.tensor_mul(ap(t["t2"]), ap(t["nk"]),
                                 ap(t["s0"])).then_inc(spool, 1)       # ->7

        @block.sync
        def _(sync):
            sync.dma_start(ap(t["adj_t"]), adj_d.ap()).then_inc(dadj, 16)
            sync.dma_start(ap(t["brow_t"]), b_d.ap()).then_inc(db, 16)
            sync.dma_start(ap(t["xidx_t"]), xidx_d.ap()).then_inc(didx, 16)
            sync.wait_ge(sfin, 2)
            sync.dma_start(out_d.ap(), ap(t["out_t"])).then_inc(dout, 16)
            sync.wait_ge(dout, 16)

        @block.scalar
        def _(act):
            act.dma_start(ap(t["xb_t"]), xb_d.ap()).then_inc(dxb, 16)
            act.dma_start(
                t["wcat_t"].ap().rearrange("p (s u) -> p s u", s=2),
                w_d.ap().rearrange("(s c) u -> c s u", s=2),
            ).then_inc(dwc, 16)
            act.wait_ge(sini, 1)
            # warm the activation table off the critical path
            nc.scalar.activation(out=ap(t["wscr"]), in_=ap(t["zcol"]),
                                 func=Act.Relu, bias=t["zcol"].ap()[:, 0:1])
            act.wait_ge(spe, 1)              # bc = ones x b broadcast done
            nc.scalar.activation(out=ap(t["z0"]), in_=bc.ap(), func=Act.Relu,
                                 bias=t["zcol"].ap()[:, 0:1],
                                 accum_out=t["z0sum"].ap()[:, 0:1]
                                 ).then_inc(sz0, 1)
            act.wait_ge(dxb, 16)
            act.wait_ge(sdve, 1)             # k
            nc.scalar.activation(out=t["xcat"].ap()[:, 0:C],
                                 in_=t["xb_t"].ap()[:, 0:C], func=Act.Copy,
                                 scale=t["k"].ap()[:, 0:1]
                                 ).then_inc(sact, 1)                   # ->1
            act.wait_ge(sdve, 3)             # asm1
            act.wait_ge(sact, 1)             # xk visible (self)
            nc.scalar.activation(out=t["xcat"].ap()[:, C:2 * C],
                                 in_=t["xcat"].ap()[:, 0:C], func=Act.Copy,
                                 scale=t["asm1"].ap()[:, 0:1]
                                 ).then_inc(sact, 1)                   # ->2
            act.wait_ge(spool, 5)            # h0 (all of iota..h1)
            nc.scalar.activation(out=ap(t["z0h"]), in_=ap(t["z0"]),
                                 func=Act.Copy, scale=t["h0"].ap()[:, 0:1]
                                 ).then_inc(sact, 1)                   # ->3
            act.wait_ge(sdve, 10)            # rn
            act.wait_ge(sact, 3)             # xcat fully visible
            nc.scalar.activation(out=t["out_t"].ap()[:, U:OUTF],
                                 in_=ap(t["xcat"]), func=Act.Copy,
                                 scale=t["rn"].ap()[:, 0:1]
                                 ).then_inc(sfin, 1)

        @block.tensor
        def _(pe):
            pe.wait_ge(sini, 1)              # ones1 ready
            pe.wait_ge(db, 16)               # b landed
            nc.tensor.matmul(bc.ap(), lhsT=t["ones1"].ap(),
                             rhs=ap(t["brow_t"]), start=True,
                             stop=True).then_inc(spe, 1)
            pe.wait_ge(dxb, 16)
            pe.wait_ge(dwc, 16)
            nc.tensor.matmul(mm.ap(), lhsT=t["xb_t"].ap()[:, C:2 * C],
                             rhs=t["wcat_t"].ap(), start=True,
                             stop=True).then_inc(spe, 1)               # ->2

        @block.vector
        def _(dve):
            dve.wait_ge(dadj, 16)
            nc.vector.reduce_sum(ap(t["k"]), ap(t["adj_t"]),
                                 axis=AX).then_inc(sdve, 1)            # ->1
            dve.wait_ge(spool, 2)            # iota + xidx_f
            nc.vector.scalar_tensor_tensor(
                out=ap(t["scr"]), in0=ap(t["iota_f"]),
                scalar=t["xidx_f"].ap()[:, 0:1], in1=ap(t["adj_t"]),
                op0=Alu.is_equal, op1=Alu.mult,
                accum_out=t["a_sel"].ap()[:, 0:1]).then_inc(sdve, 1)   # ->2
            dve.wait_ge(sdve, 2)             # a_sel accum lands async
            nc.vector.tensor_scalar(out=ap(t["asm1"]), in0=ap(t["a_sel"]),
                                    scalar1=-1.0, scalar2=None,
                                    op0=Alu.add).then_inc(sdve, 1)     # ->3
            dve.wait_ge(spe, 2)              # mm = [u | v]
            dve.wait_ge(sdve, 3)             # asm1 visible
            nc.vector.tensor_scalar(out=ap(t["t_sb"]),
                                    in0=mm.ap()[:, U:2 * U],
                                    scalar1=t["asm1"].ap()[:, 0:1],
                                    scalar2=None,
                                    op0=Alu.mult).then_inc(sdve, 1)    # ->4
            dve.wait_ge(sdve, 4)             # t_sb visible
            nc.vector.tensor_add(ap(t["zz"]), ap(t["t_sb"]),
                                 mm.ap()[:, 0:U]).then_inc(sdve, 1)    # ->5
            dve.wait_ge(sdve, 5)             # zz visible
            nc.vector.tensor_add(ap(t["zzb"]), ap(t["zz"]),
                                 bc.ap()).then_inc(sdve, 1)            # ->6
            dve.wait_ge(sdve, 6)             # zzb visible
            nc.vector.tensor_scalar(out=ap(t["z1"]), in0=ap(t["zzb"]),
                                    scalar1=0.0, scalar2=None, op0=Alu.max,
                                    op1=Alu.add,
                                    accum_out=t["z1sum"].ap()[:, 0:1]
                                    ).then_inc(sdve, 1)                # ->7
            dve.wait_ge(sdve, 7)             # z1sum accum landed
            nc.vector.tensor_scalar(out=ap(t["s1"]), in0=ap(t["z1sum"]),
                                    scalar1=0.0, scalar2=None,
                                    op0=Alu.is_gt).then_inc(sdve, 1)   # ->8
            dve.wait_ge(spool, 7)            # t2
            dve.wait_ge(sdve, 8)             # s1 visible
            nc.vector.scalar_tensor_tensor(
                out=ap(t["nn"]), in0=ap(t["k"]),
                scalar=t["s1"].ap()[:, 0:1], in1=ap(t["t2"]),
                op0=Alu.mult, op1=Alu.add).then_inc(sdve, 1)           # ->9
            dve.wait_ge(sdve, 9)             # nn visible
            nc.vector.reciprocal(ap(t["rn"]),
                                 ap(t["nn"])).then_inc(sdve, 1)        # ->10
            dve.wait_ge(sact, 3)             # z0h
            nc.vector.scalar_tensor_tensor(
                out=t["out_t"].ap()[:, 0:U], in0=ap(t["z1"]),
                scalar=t["h1"].ap()[:, 0:1], in1=ap(t["z0h"]),
                op0=Alu.mult, op1=Alu.max).then_inc(sfin, 1)

    nc.compile()
    return nc


def get_nc():
    if "nc" not in _CACHE:
        _CACHE["nc"] = _build_nc()
    return _CACHE["nc"]


def make_in_maps(inputs, adj_matrix, xidx, w, b):
    """Shard full inputs into per-core input maps (128 (b,i) rows per core)."""
    x_flat = np.asarray(inputs, dtype=np.float32).reshape(B * N, C)
    adj_flat = np.ascontiguousarray(
        np.asarray(adj_matrix, dtype=np.float32).reshape(B * N, N))
    xidx_flat = np.ascontiguousarray(
        np.asarray(xidx, dtype=np.int32).reshape(B * N, 1))
    w_full = np.ascontiguousarray(np.asarray(w, dtype=np.float32)[0])
    b_full = np.ascontiguousarray(
        np.asarray(b, dtype=np.float32).reshape(1, U))

    in_maps = []
    for c in range(NCORES):
        rows = slice(c * P, (c + 1) * P)
        x_slab = x_flat[rows]
        in_maps.append({
            "adj": adj_flat[rows],
            "xboth": np.ascontiguousarray(
                np.concatenate([x_slab, x_slab.T], axis=1)),
            "xidx": xidx_flat[rows],
            "w": w_full,
            "b": b_full,
        })
    return in_maps


def kernel(inputs, adj_matrix, xidx, w, b, _trace=False):
    from concourse.bass_utils import run_bass_kernel_spmd

    nc = get_nc()
    in_maps = make_in_maps(inputs, adj_matrix, xidx, w, b)
    res = run_bass_kernel_spmd(nc, in_maps, list(range(NCORES)),
                               trace=_trace)
    out = np.concatenate([res.results[c]["out"] for c in range(NCORES)],
                         axis=0)
    out = out.reshape(B, N, OUTF).astype(np.float32)
    if _trace:
        _CACHE["last_results"] = res
    return out
